# revision 18
# baseline (speedup 1.0000x reference)
"""Single-head self-attention (B=4, S=2048, D=1024) on 8 trn2 NeuronCores.

Sharding: core c -> (batch b = c//2, query half h = c%2). Each core gets a
permuted x^T for its batch (own seq-half first; softmax is invariant to key
permutation), computes Q for its 1024 queries and K/V for all 2048 keys,
then attention. Output rows are the core's own queries in original order, so
the host gather is a pure concatenation.

Device layout (per core):
  xt (input)  : [1024_d, 2048_t] fp32   (x_perm.T, host-prepared)
  Q^T         : [128_dp, 8_dc, 512_s]   per s-block, SBUF
  K^T         : [128_jp, 8_jc, 2048_t]  spilled to HBM scratch, streamed back
  V           : [128_tp, 16_tc, 1024_j] SBUF-resident
  scores^T    : [128_t, 512_s] PSUM -> exp -> SBUF (keys on partitions, so
                attn@V needs no transposes; softmax sum over keys is an
                extra N=1 matmul vs a ones vector sharing the stationary
                operand; max-subtraction skipped: scores ~ N(0, 0.33))
All matmuls fp32r (fp22 mantissa, full PE rate at N>=256, fp32 accumulate).
"""

import os
import sys
import types

import numpy as np

B, S, D = 4, 2048, 1024
HALF = S // 2  # 1024 queries per core
SCALE = 1.0 / 32.0  # 1/sqrt(D)
NC = 8
DC = D // 128  # 8 d-chunks
TT = S // 128  # 16 key tiles
SBLK = 512  # queries per s-block
NSB = HALF // SBLK  # 2 s-blocks

_CACHED_NC = None
LAST_RESULT = None  # BassKernelResults of the most recent run (for test.py)


def _ensure_axon_ntff_hook():
    """bass_utils' trace path needs antenv.axon_hooks; this image's antenv
    lacks it. Install a shim backed by trn_agent_boot's ctypes hook so
    BASS_TRACE=1 profiling works. No-op if already present/unavailable."""
    try:
        import antenv.axon_hooks  # noqa: F401

        return
    except ImportError:
        pass
    try:
        from trn_agent_boot.trn_boot import _ntff_profile_via_ctypes

        hook = _ntff_profile_via_ctypes("/opt/axon/libaxon_pjrt.so")
    except Exception:
        hook = None
    mod = types.ModuleType("antenv.axon_hooks")
    mod.get_axon_ntff_profile_hook = lambda: hook
    mod.set_axon_ntff_profile_hook = lambda h: None
    sys.modules["antenv.axon_hooks"] = mod


def build_kernel(tc, xt, xn, wq, wk, wv, bq, bv, out):
    """Per-core attention with neither K^T nor V materialized:
      Q^T = Wq-proj of own queries (+bq)            [128, DC, 1024_s]
      G   = Wk @ Q^T   (K-side projection applied to the small Q side)
      scores^T[t, s] = sum_d xT[d, t] G[d, s]       (K bias cancels)
      H^T[d, s] = sum_t x[t, d] expP[t, s]          (attn contracts x first)
      out[s, j] = (sum_d H^T[d, s] Wv[d, j]) / l[s] + bv[j]
    This removes every duplicated projection: 15.05 GFLOP/core, the exact
    1/8 share of the network's total work.
    """
    import concourse.bass as bass
    from concourse import mybir

    nc = tc.nc
    F32 = mybir.dt.float32
    F32R = mybir.dt.float32r
    Identity = mybir.ActivationFunctionType.Identity
    Copy = mybir.ActivationFunctionType.Copy
    Exp = mybir.ActivationFunctionType.Exp

    xt_r = xt.rearrange("(c p) t -> p c t", p=128)  # [128, 8, 2048]
    xn_r = xn.rearrange("(tc p) d -> p tc d", p=128)  # [128, 16, 1024]
    out_r = out.rearrange("(su p) j -> su p j", p=128)  # [8, 128, 1024]

    with tc.tile_pool(name="persist", bufs=1) as persist:
        xT = persist.tile([128, DC, S], F32R)
        G = persist.tile([128, DC, HALF], F32R)
        bv_bc = persist.tile([128, D], F32)
        bv_bcast_ap = bass.AP(
            tensor=bv.tensor, offset=bv.offset, ap=[[0, 128]] + list(bv.ap)
        )
        nc.scalar.dma_start(bv_bc, bv_bcast_ap)
        bq_sb = persist.tile([128, DC], F32)
        nc.scalar.dma_start(bq_sb, bq)
        ones_f = persist.tile([128, 2], F32)
        nc.vector.memset(ones_f, 1.0)
        ones_t = persist.tile([128, 2], F32R)
        nc.vector.tensor_copy(ones_t, ones_f)

        # x^T loaded t-block-major so early query groups unblock first
        for tb in range(S // 512):
            for c in range(DC):
                nc.sync.dma_start(
                    xT[:, c, tb * 512 : (tb + 1) * 512],
                    xt_r[:, c, tb * 512 : (tb + 1) * 512],
                )

        # ---- Phase A: Q^T then G = Wk @ Q^T ------------------------------
        with (
            tc.tile_pool(name="pa1", bufs=1) as pa1,
            tc.tile_pool(name="pa_w", bufs=8) as paw,
            tc.tile_pool(name="psa", bufs=2, space="PSUM") as psa,
        ):
            qT = pa1.tile([128, DC, HALF], F32R)
            for qc in range(DC):
                wq_t = paw.tile([128, DC, 128], F32R, tag="w_t")
                nc.scalar.dma_start(wq_t, wq[:, :, qc * 128 : (qc + 1) * 128])
                for sblk in range(NSB):
                    qpsum = psa.tile([128, SBLK], F32, tag="qpsum")
                    for c in range(DC):
                        nc.tensor.matmul(
                            qpsum,
                            wq_t[:, c, :],
                            xT[:, c, sblk * SBLK : (sblk + 1) * SBLK],
                            start=(c == 0),
                            stop=(c == DC - 1),
                        )
                    nc.scalar.activation(
                        qT[:, qc, sblk * SBLK : (sblk + 1) * SBLK],
                        qpsum,
                        Identity,
                        bias=bq_sb[:, qc : qc + 1],
                    )

            # G[d, s] = sum_j Wk[d, j] qT[j, s]  (wk passed j-major = Wk.T)
            for gc in range(DC):
                wk_t = paw.tile([128, DC, 128], F32R, tag="w_t")
                nc.scalar.dma_start(wk_t, wk[:, :, gc * 128 : (gc + 1) * 128])
                for sblk in range(NSB):
                    gpsum = psa.tile([128, SBLK], F32, tag="gpsum")
                    for jc in range(DC):
                        nc.tensor.matmul(
                            gpsum,
                            wk_t[:, jc, :],
                            qT[:, jc, sblk * SBLK : (sblk + 1) * SBLK],
                            start=(jc == 0),
                            stop=(jc == DC - 1),
                        )
                    nc.scalar.activation(
                        G[:, gc, sblk * SBLK : (sblk + 1) * SBLK], gpsum, Copy
                    )

        # ---- Phase B: scores^T -> exp -> H^T -> out, per 512-query block -
        with (
            tc.tile_pool(name="pb_wv", bufs=1) as pbwv,
            tc.tile_pool(name="pb_p", bufs=1) as pbp,
            tc.tile_pool(name="pb_x", bufs=2) as pbx,
            tc.tile_pool(name="pb_h", bufs=1) as pbh,
            tc.tile_pool(name="pb_o", bufs=2) as pbo,
            tc.tile_pool(name="pb_m", bufs=2) as pbm,
            tc.tile_pool(name="psb_s", bufs=3, space="PSUM") as psbs,
            tc.tile_pool(name="psb_h", bufs=2, space="PSUM") as psbh,
            tc.tile_pool(name="psb_o", bufs=2, space="PSUM") as psbo,
            tc.tile_pool(name="psb_l", bufs=1, space="PSUM") as psbl,
        ):
            wv_sb = pbwv.tile([128, DC, D], F32R)
            nc.gpsimd.dma_start(wv_sb, wv)
            for sb in range(NSB):
                # scores^T + exp; E accumulates the softmax sums on DVE
                expP = pbp.tile([128, TT, SBLK], F32R, tag="expP")
                E_t = pbp.tile([128, SBLK], F32R, tag="E_t", bufs=1)
                for tt in range(TT):
                    spsum = psbs.tile([128, SBLK], F32, tag="spsum")
                    for c in range(DC):
                        nc.tensor.matmul(
                            spsum,
                            xT[:, c, tt * 128 : (tt + 1) * 128],
                            G[:, c, sb * SBLK : (sb + 1) * SBLK],
                            start=(c == 0),
                            stop=(c == DC - 1),
                        )
                    nc.scalar.activation(expP[:, tt, :], spsum, Exp, scale=SCALE)
                    if tt == 1:
                        nc.vector.tensor_add(E_t, expP[:, 0, :], expP[:, 1, :])
                    elif tt > 1:
                        nc.vector.tensor_add(E_t, E_t, expP[:, tt, :])

                # H^T[d, s] = sum_t x[t, d] expP[t, s]
                H = pbh.tile([128, DC, SBLK], F32R, tag="H")
                for dc in range(DC):
                    xn_t = pbx.tile([128, TT, 128], F32R, tag="xn_t")
                    nc.sync.dma_start(xn_t, xn_r[:, :, dc * 128 : (dc + 1) * 128])
                    hpsum = psbh.tile([128, SBLK], F32, tag="hpsum")
                    for tt in range(TT):
                        nc.tensor.matmul(
                            hpsum,
                            xn_t[:, tt, :],
                            expP[:, tt, :],
                            start=(tt == 0),
                            stop=(tt == TT - 1),
                        )
                    nc.scalar.activation(H[:, dc, :], hpsum, Copy)

                # out[s, j] = (sum_d H^T[d, s] Wv[d, j]) / l[s] + bv[j]
                for su in range(SBLK // 128):
                    s0 = su * 128
                    lpsum = psbl.tile([128, 2], F32, tag="lpsum")
                    nc.tensor.matmul(
                        lpsum, E_t[:, s0 : s0 + 128], ones_t, start=True, stop=True
                    )
                    recip = pbm.tile([128, 1], F32, tag="recip")
                    nc.vector.reciprocal(recip, lpsum[:, 0:1])
                    for jb in range(2):
                        opsum = psbo.tile([128, 512], F32, tag="opsum")
                        for dc in range(DC):
                            nc.tensor.matmul(
                                opsum,
                                H[:, dc, s0 : s0 + 128],
                                wv_sb[:, dc, jb * 512 : (jb + 1) * 512],
                                start=(dc == 0),
                                stop=(dc == DC - 1),
                            )
                        o_sb = pbo.tile([128, 512], F32, tag="o_sb")
                        nc.vector.tensor_scalar_mul(o_sb, in0=opsum, scalar1=recip)
                        nc.vector.tensor_add(
                            o_sb, o_sb, bv_bc[:, jb * 512 : (jb + 1) * 512]
                        )
                        nc.sync.dma_start(
                            out_r[sb * (SBLK // 128) + su][
                                :, jb * 512 : (jb + 1) * 512
                            ],
                            o_sb,
                        )


def build_nc():
    global _CACHED_NC
    if _CACHED_NC is not None:
        return _CACHED_NC
    import concourse.tile as tile
    from concourse import bacc, mybir

    F32 = mybir.dt.float32
    F32R = mybir.dt.float32r
    nc = bacc.Bacc("TRN2", target_bir_lowering=False, debug=False)
    xt = nc.dram_tensor("xt", [D, S], F32R, kind="ExternalInput").ap()
    xn = nc.dram_tensor("xn", [S, D], F32R, kind="ExternalInput").ap()
    wq = nc.dram_tensor("wq", [128, DC, D], F32R, kind="ExternalInput").ap()
    wk = nc.dram_tensor("wk", [128, DC, D], F32R, kind="ExternalInput").ap()
    wv = nc.dram_tensor("wv", [128, DC, D], F32R, kind="ExternalInput").ap()
    bq = nc.dram_tensor("bq", [128, DC], F32, kind="ExternalInput").ap()
    bv = nc.dram_tensor("bv", [D], F32, kind="ExternalInput").ap()
    out = nc.dram_tensor("out", [HALF, D], F32, kind="ExternalOutput").ap()

    with tile.TileContext(nc) as tc:
        build_kernel(tc, xt, xn, wq, wk, wv, bq, bv, out)
    nc.compile()
    _CACHED_NC = nc
    return nc


def _shard_inputs(x, Wq, bq, Wk, bk, Wv, bv):
    """Host-side prep: per-core permuted x^T + relaid-out weights/biases."""
    wq_r = np.ascontiguousarray(Wq.reshape(DC, 128, D).transpose(1, 0, 2))
    wk_r = np.ascontiguousarray(Wk.T.reshape(DC, 128, D).transpose(1, 0, 2))
    wv_r = np.ascontiguousarray(Wv.reshape(DC, 128, D).transpose(1, 0, 2))
    bq_r = np.ascontiguousarray(bq.reshape(DC, 128).T)
    bv_c = np.ascontiguousarray(bv)

    in_maps = []
    for c in range(NC):
        b, h = divmod(c, 2)
        xb = x[b]
        if h:
            xb = np.concatenate([xb[HALF:], xb[:HALF]], axis=0)
        xt = np.ascontiguousarray(xb.T)  # [D, S], own queries first
        xn = np.ascontiguousarray(xb)  # [S, D], same permutation
        in_maps.append(
            {
                "xt": xt,
                "xn": xn,
                "wq": wq_r,
                "wk": wk_r,
                "wv": wv_r,
                "bq": bq_r,
                "bv": bv_c,
            }
        )
    return in_maps


def kernel(x, Wq, bq, Wk, bk, Wv, bv):
    global LAST_RESULT
    _ensure_axon_ntff_hook()
    from concourse import bass_utils

    x = np.asarray(x, dtype=np.float32)
    args = [np.asarray(a, dtype=np.float32) for a in (Wq, bq, Wk, bk, Wv, bv)]
    nc = build_nc()
    in_maps = _shard_inputs(x, *args)
    res = bass_utils.run_bass_kernel_spmd(nc, in_maps, core_ids=list(range(NC)))
    LAST_RESULT = res
    out = np.empty((B, S, D), dtype=np.float32)
    for c in range(NC):
        b, h = divmod(c, 2)
        out[b, h * HALF : (h + 1) * HALF, :] = res.results[c]["out"]
    return out


if __name__ == "__main__":
    rng = np.random.default_rng(0)
    init = 1.0 / 32.0
    x = rng.standard_normal((B, S, D), dtype=np.float32)
    mk = lambda *s: rng.uniform(-init, init, s).astype(np.float32)
    o = kernel(x, mk(D, D), mk(D), mk(D, D), mk(D), mk(D, D), mk(D))
    print("out", o.shape, o.dtype, float(np.abs(o).max()))


# revision 19
# speedup vs baseline: 1.0011x; 1.0011x over previous
"""Single-head self-attention (B=4, S=2048, D=1024) on 8 trn2 NeuronCores.

Sharding: core c -> (batch b = c//2, query half h = c%2). Each core gets a
permuted x^T for its batch (own seq-half first; softmax is invariant to key
permutation), computes Q for its 1024 queries and K/V for all 2048 keys,
then attention. Output rows are the core's own queries in original order, so
the host gather is a pure concatenation.

Device layout (per core):
  xt (input)  : [1024_d, 2048_t] fp32   (x_perm.T, host-prepared)
  Q^T         : [128_dp, 8_dc, 512_s]   per s-block, SBUF
  K^T         : [128_jp, 8_jc, 2048_t]  spilled to HBM scratch, streamed back
  V           : [128_tp, 16_tc, 1024_j] SBUF-resident
  scores^T    : [128_t, 512_s] PSUM -> exp -> SBUF (keys on partitions, so
                attn@V needs no transposes; softmax sum over keys is an
                extra N=1 matmul vs a ones vector sharing the stationary
                operand; max-subtraction skipped: scores ~ N(0, 0.33))
All matmuls fp32r (fp22 mantissa, full PE rate at N>=256, fp32 accumulate).
"""

import os
import sys
import types

import numpy as np

B, S, D = 4, 2048, 1024
HALF = S // 2  # 1024 queries per core
SCALE = 1.0 / 32.0  # 1/sqrt(D)
NC = 8
DC = D // 128  # 8 d-chunks
TT = S // 128  # 16 key tiles
SBLK = 512  # queries per s-block
NSB = HALF // SBLK  # 2 s-blocks

_CACHED_NC = None
LAST_RESULT = None  # BassKernelResults of the most recent run (for test.py)


def _ensure_axon_ntff_hook():
    """bass_utils' trace path needs antenv.axon_hooks; this image's antenv
    lacks it. Install a shim backed by trn_agent_boot's ctypes hook so
    BASS_TRACE=1 profiling works. No-op if already present/unavailable."""
    try:
        import antenv.axon_hooks  # noqa: F401

        return
    except ImportError:
        pass
    try:
        from trn_agent_boot.trn_boot import _ntff_profile_via_ctypes

        hook = _ntff_profile_via_ctypes("/opt/axon/libaxon_pjrt.so")
    except Exception:
        hook = None
    mod = types.ModuleType("antenv.axon_hooks")
    mod.get_axon_ntff_profile_hook = lambda: hook
    mod.set_axon_ntff_profile_hook = lambda h: None
    sys.modules["antenv.axon_hooks"] = mod


def build_kernel(tc, xt, xn, wq, wk, wv, bq, bv, out):
    """Per-core attention with neither K^T nor V materialized:
      Q^T = Wq-proj of own queries (+bq)            [128, DC, 1024_s]
      G   = Wk @ Q^T   (K-side projection applied to the small Q side)
      scores^T[t, s] = sum_d xT[d, t] G[d, s]       (K bias cancels)
      H^T[d, s] = sum_t x[t, d] expP[t, s]          (attn contracts x first)
      out[s, j] = (sum_d H^T[d, s] Wv[d, j]) / l[s] + bv[j]
    This removes every duplicated projection: 15.05 GFLOP/core, the exact
    1/8 share of the network's total work.
    """
    import concourse.bass as bass
    from concourse import mybir

    nc = tc.nc
    F32 = mybir.dt.float32
    F32R = mybir.dt.float32r
    Identity = mybir.ActivationFunctionType.Identity
    Copy = mybir.ActivationFunctionType.Copy
    Exp = mybir.ActivationFunctionType.Exp

    xt_r = xt.rearrange("(c p) t -> p c t", p=128)  # [128, 8, 2048]
    xn_r = xn.rearrange("(tc p) d -> p tc d", p=128)  # [128, 16, 1024]
    out_r = out.rearrange("(su p) j -> su p j", p=128)  # [8, 128, 1024]

    with tc.tile_pool(name="persist", bufs=1) as persist:
        xT = persist.tile([128, DC, S], F32R)
        G = persist.tile([128, DC, HALF], F32R)
        bv_bc = persist.tile([128, D], F32)
        bv_bcast_ap = bass.AP(
            tensor=bv.tensor, offset=bv.offset, ap=[[0, 128]] + list(bv.ap)
        )
        nc.scalar.dma_start(bv_bc, bv_bcast_ap)
        bq_sb = persist.tile([128, DC], F32)
        nc.scalar.dma_start(bq_sb, bq)
        ones_f = persist.tile([128, 2], F32)
        nc.vector.memset(ones_f, 1.0)
        ones_t = persist.tile([128, 2], F32R)
        nc.vector.tensor_copy(ones_t, ones_f)

        # x^T loaded t-block-major so early query groups unblock first
        for tb in range(S // 512):
            for c in range(DC):
                nc.sync.dma_start(
                    xT[:, c, tb * 512 : (tb + 1) * 512],
                    xt_r[:, c, tb * 512 : (tb + 1) * 512],
                )

        # ---- Phase A: Q^T then G = Wk @ Q^T ------------------------------
        with (
            tc.tile_pool(name="pa1", bufs=1) as pa1,
            tc.tile_pool(name="pa_w", bufs=6) as paw,
            tc.tile_pool(name="psa", bufs=2, space="PSUM") as psa,
        ):
            qT = pa1.tile([128, DC, HALF], F32R)
            for qc in range(DC):
                wq_t = paw.tile([128, DC, 128], F32R, tag="w_t")
                nc.scalar.dma_start(wq_t, wq[:, :, qc * 128 : (qc + 1) * 128])
                for sblk in range(NSB):
                    qpsum = psa.tile([128, SBLK], F32, tag="qpsum")
                    for c in range(DC):
                        nc.tensor.matmul(
                            qpsum,
                            wq_t[:, c, :],
                            xT[:, c, sblk * SBLK : (sblk + 1) * SBLK],
                            start=(c == 0),
                            stop=(c == DC - 1),
                        )
                    nc.scalar.activation(
                        qT[:, qc, sblk * SBLK : (sblk + 1) * SBLK],
                        qpsum,
                        Identity,
                        bias=bq_sb[:, qc : qc + 1],
                    )

            # G[d, s] = sum_j Wk[d, j] qT[j, s]  (wk passed j-major = Wk.T)
            for gc in range(DC):
                wk_t = paw.tile([128, DC, 128], F32R, tag="w_t")
                nc.scalar.dma_start(wk_t, wk[:, :, gc * 128 : (gc + 1) * 128])
                for sblk in range(NSB):
                    gpsum = psa.tile([128, SBLK], F32, tag="gpsum")
                    for jc in range(DC):
                        nc.tensor.matmul(
                            gpsum,
                            wk_t[:, jc, :],
                            qT[:, jc, sblk * SBLK : (sblk + 1) * SBLK],
                            start=(jc == 0),
                            stop=(jc == DC - 1),
                        )
                    nc.scalar.activation(
                        G[:, gc, sblk * SBLK : (sblk + 1) * SBLK], gpsum, Copy
                    )

        # ---- Phase B: scores^T -> exp -> H^T -> out, per 512-query block -
        with (
            tc.tile_pool(name="pb_wv", bufs=1) as pbwv,
            tc.tile_pool(name="pb_p", bufs=1) as pbp,
            tc.tile_pool(name="pb_x", bufs=2) as pbx,
            tc.tile_pool(name="pb_h", bufs=1) as pbh,
            tc.tile_pool(name="pb_o", bufs=2) as pbo,
            tc.tile_pool(name="pb_m", bufs=2) as pbm,
            tc.tile_pool(name="psb_s", bufs=2, space="PSUM") as psbs,
            tc.tile_pool(name="psb_h", bufs=2, space="PSUM") as psbh,
            tc.tile_pool(name="psb_o", bufs=2, space="PSUM") as psbo,
            tc.tile_pool(name="psb_l", bufs=2, space="PSUM") as psbl,
        ):
            wv_sb = pbwv.tile([128, DC, D], F32R)
            nc.gpsimd.dma_start(wv_sb, wv)
            for sb in range(NSB):
                # scores^T + exp; E accumulates the softmax sums on DVE
                expP = pbp.tile([128, TT, SBLK], F32R, tag="expP")
                E_t = pbp.tile([128, SBLK], F32R, tag="E_t", bufs=1)
                for tt in range(TT):
                    spsum = psbs.tile([128, SBLK], F32, tag="spsum")
                    for c in range(DC):
                        nc.tensor.matmul(
                            spsum,
                            xT[:, c, tt * 128 : (tt + 1) * 128],
                            G[:, c, sb * SBLK : (sb + 1) * SBLK],
                            start=(c == 0),
                            stop=(c == DC - 1),
                        )
                    nc.scalar.activation(expP[:, tt, :], spsum, Exp, scale=SCALE)
                    if tt == 1:
                        nc.vector.tensor_add(E_t, expP[:, 0, :], expP[:, 1, :])
                    elif tt > 1:
                        nc.vector.tensor_add(E_t, E_t, expP[:, tt, :])

                # H^T[d, s] = sum_t x[t, d] expP[t, s]
                H = pbh.tile([128, DC, SBLK], F32R, tag="H")
                for dc in range(DC):
                    xn_t = pbx.tile([128, TT, 128], F32R, tag="xn_t")
                    nc.sync.dma_start(xn_t, xn_r[:, :, dc * 128 : (dc + 1) * 128])
                    hpsum = psbh.tile([128, SBLK], F32, tag="hpsum")
                    for tt in range(TT):
                        nc.tensor.matmul(
                            hpsum,
                            xn_t[:, tt, :],
                            expP[:, tt, :],
                            start=(tt == 0),
                            stop=(tt == TT - 1),
                        )
                    nc.scalar.activation(H[:, dc, :], hpsum, Copy)

                # out[s, j] = (sum_d H^T[d, s] Wv[d, j]) / l[s] + bv[j]
                for su in range(SBLK // 128):
                    s0 = su * 128
                    lpsum = psbl.tile([128, 2], F32, tag="lpsum")
                    nc.tensor.matmul(
                        lpsum, E_t[:, s0 : s0 + 128], ones_t, start=True, stop=True
                    )
                    recip = pbm.tile([128, 1], F32, tag="recip")
                    nc.vector.reciprocal(recip, lpsum[:, 0:1])
                    for jb in range(2):
                        opsum = psbo.tile([128, 512], F32, tag="opsum")
                        for dc in range(DC):
                            nc.tensor.matmul(
                                opsum,
                                H[:, dc, s0 : s0 + 128],
                                wv_sb[:, dc, jb * 512 : (jb + 1) * 512],
                                start=(dc == 0),
                                stop=(dc == DC - 1),
                            )
                        o_sb = pbo.tile([128, 512], F32, tag="o_sb")
                        nc.vector.tensor_scalar_mul(o_sb, in0=opsum, scalar1=recip)
                        nc.vector.tensor_add(
                            o_sb, o_sb, bv_bc[:, jb * 512 : (jb + 1) * 512]
                        )
                        nc.sync.dma_start(
                            out_r[sb * (SBLK // 128) + su][
                                :, jb * 512 : (jb + 1) * 512
                            ],
                            o_sb,
                        )


def build_nc():
    global _CACHED_NC
    if _CACHED_NC is not None:
        return _CACHED_NC
    import concourse.tile as tile
    from concourse import bacc, mybir

    F32 = mybir.dt.float32
    F32R = mybir.dt.float32r
    nc = bacc.Bacc("TRN2", target_bir_lowering=False, debug=False)
    xt = nc.dram_tensor("xt", [D, S], F32R, kind="ExternalInput").ap()
    xn = nc.dram_tensor("xn", [S, D], F32R, kind="ExternalInput").ap()
    wq = nc.dram_tensor("wq", [128, DC, D], F32R, kind="ExternalInput").ap()
    wk = nc.dram_tensor("wk", [128, DC, D], F32R, kind="ExternalInput").ap()
    wv = nc.dram_tensor("wv", [128, DC, D], F32R, kind="ExternalInput").ap()
    bq = nc.dram_tensor("bq", [128, DC], F32, kind="ExternalInput").ap()
    bv = nc.dram_tensor("bv", [D], F32, kind="ExternalInput").ap()
    out = nc.dram_tensor("out", [HALF, D], F32, kind="ExternalOutput").ap()

    with tile.TileContext(nc) as tc:
        build_kernel(tc, xt, xn, wq, wk, wv, bq, bv, out)
    nc.compile()
    _CACHED_NC = nc
    return nc


def _shard_inputs(x, Wq, bq, Wk, bk, Wv, bv):
    """Host-side prep: per-core permuted x^T + relaid-out weights/biases."""
    wq_r = np.ascontiguousarray(Wq.reshape(DC, 128, D).transpose(1, 0, 2))
    wk_r = np.ascontiguousarray(Wk.T.reshape(DC, 128, D).transpose(1, 0, 2))
    wv_r = np.ascontiguousarray(Wv.reshape(DC, 128, D).transpose(1, 0, 2))
    bq_r = np.ascontiguousarray(bq.reshape(DC, 128).T)
    bv_c = np.ascontiguousarray(bv)

    in_maps = []
    for c in range(NC):
        b, h = divmod(c, 2)
        xb = x[b]
        if h:
            xb = np.concatenate([xb[HALF:], xb[:HALF]], axis=0)
        xt = np.ascontiguousarray(xb.T)  # [D, S], own queries first
        xn = np.ascontiguousarray(xb)  # [S, D], same permutation
        in_maps.append(
            {
                "xt": xt,
                "xn": xn,
                "wq": wq_r,
                "wk": wk_r,
                "wv": wv_r,
                "bq": bq_r,
                "bv": bv_c,
            }
        )
    return in_maps


def kernel(x, Wq, bq, Wk, bk, Wv, bv):
    global LAST_RESULT
    _ensure_axon_ntff_hook()
    from concourse import bass_utils

    x = np.asarray(x, dtype=np.float32)
    args = [np.asarray(a, dtype=np.float32) for a in (Wq, bq, Wk, bk, Wv, bv)]
    nc = build_nc()
    in_maps = _shard_inputs(x, *args)
    res = bass_utils.run_bass_kernel_spmd(nc, in_maps, core_ids=list(range(NC)))
    LAST_RESULT = res
    out = np.empty((B, S, D), dtype=np.float32)
    for c in range(NC):
        b, h = divmod(c, 2)
        out[b, h * HALF : (h + 1) * HALF, :] = res.results[c]["out"]
    return out


if __name__ == "__main__":
    rng = np.random.default_rng(0)
    init = 1.0 / 32.0
    x = rng.standard_normal((B, S, D), dtype=np.float32)
    mk = lambda *s: rng.uniform(-init, init, s).astype(np.float32)
    o = kernel(x, mk(D, D), mk(D), mk(D, D), mk(D), mk(D, D), mk(D))
    print("out", o.shape, o.dtype, float(np.abs(o).max()))
